# revision 5
# baseline (speedup 1.0000x reference)
"""BertEmbedding (scalar-mix + ragged mean-pool + projection) on 8 TRN2 cores.

Full-input contract: kernel(**inputs) takes the unsharded numpy inputs and
returns the full [32, 256, 400] f32 output. Internally: data-parallel over
batch (4 examples per core), proj_w replicated (pre-transposed on host). All
math from inputs to outputs runs on-device; the host only shards/relayouts
and inspects inputs to pick/specialize the program variant (cached per key).

Structural choices (v3):
  - Ragged bound: positions p >= sum(bert_lens[b]) fall in the reference's
    overflow bucket and contribute nothing, so only T = max_b sum(lens[b])
    subword rows are loaded per example (T read from the runtime input).
  - Equal-mix fast path: when all mix_weights entries are equal (softmax is
    exactly uniform, as in the spec's fill=zeros), sum_l w_l*hid_l =
    w_0 * sum_l hid_l. The layer sum is formed by DMA accumulation (SWDGE
    accum_op=add) into two SBUF lanes per example (A: l0+l2, B: l1+l3) with
    zero compute-engine work, and the shared scale w_0 = gamma*softmax[0]
    folds into the membership build's spare tensor_scalar op slot.
  - Pooling matmul: pooledT[h, j] += lane[p, h] * M[p, j] with the scaled
    membership M as rhs, lane chunks as PE weights, accumulating both lanes
    and all position chunks into per-h-slice PSUM banks.
  - Column banding: a position p can only pool into words j with
    ends[j] >= p+1 and ends[j] <= Lmax*(j+1), so chunk c only needs columns
    j >= ceil((128c+1)/Lmax)-1 (Lmax = max(bert_lens), host-computed).
  - Projection in bf16 (full PE rate; contraction error ~1e-3 << 2e-2 tol)
    with the 1/cnt scale applied as per-partition ACT scale on PSUM copy.
  - General-weights fallback: DVE premix (tensor_scalar + 3
    scalar_tensor_tensor passes) of the 4 layers, then the same pooling.

Matmuls run in f32r (membership/pooling) and bf16 (projection); membership
build, scans and softmax run in exact f32.

Input-spec property relied on (declared in the problem spec):
  - bert_mask fill=ones -> positions' mask cumsum is the position index.
"""

import numpy as np

NL, B, SW, H = 4, 32, 512, 768
SL, NOUT = 256, 400
NCORES = 8
BPC = B // NCORES  # examples per core
HC = H // 128      # hidden chunks
JC = SL // 128     # word chunks

_NC_CACHE = {}
LAST_RESULT = None  # BassKernelResults of the last run (for profiling)


def _build_nc(T, eq, bands):
    """Per-core program: position bound T, equal-weights flag, per-chunk
    word-band lower bounds (len == ceil(T/128))."""
    import concourse.bacc as bacc
    import concourse.tile as tile
    from concourse import mybir

    f32 = mybir.dt.float32
    f32r = mybir.dt.float32r
    bf16 = mybir.dt.bfloat16
    i32 = mybir.dt.int32
    Alu = mybir.AluOpType
    Act = mybir.ActivationFunctionType
    Axis = mybir.AxisListType

    CH = (T + 127) // 128          # position chunks
    P = T - 128 * (CH - 1)         # partitions in the last chunk
    CHf = CH if P == 128 else CH - 1  # chunks covered by the bulk DMA
    assert len(bands) == CH and bands[0] == 0

    nc = bacc.Bacc(None)
    hid = nc.dram_tensor("hid", [NL, BPC, T, H], f32r, kind="ExternalInput")
    lens = nc.dram_tensor("lens", [BPC, SL], i32, kind="ExternalInput")
    mw = nc.dram_tensor("mw", [1, NL], f32, kind="ExternalInput")
    gam = nc.dram_tensor("gam", [1, 1], f32, kind="ExternalInput")
    projT = nc.dram_tensor("projT", [H, NOUT], f32, kind="ExternalInput")
    sel = nc.dram_tensor("sel", [BPC, BPC * 128], f32, kind="ExternalInput")
    out = nc.dram_tensor("out", [BPC, SL, NOUT], f32, kind="ExternalOutput")

    with tile.TileContext(nc) as tc:
        with (
            tc.tile_pool(name="const", bufs=1) as const,
            tc.tile_pool(name="small", bufs=1) as small,
            tc.tile_pool(name="h", bufs=6 if eq else 8) as hpool,
            tc.tile_pool(name="acc", bufs=3) as accpool,
            tc.tile_pool(name="mtmp", bufs=2) as mpool,
            tc.tile_pool(name="Mm", bufs=1) as Mpool,
            tc.tile_pool(name="se", bufs=2) as sepool,
            tc.tile_pool(name="pt", bufs=2) as ptpool,
            tc.tile_pool(name="osb", bufs=2) as opool,
            tc.tile_pool(name="psb", bufs=1, space="PSUM") as ps_b,
            tc.tile_pool(name="psp", bufs=1, space="PSUM") as ps_p,
            tc.tile_pool(name="pso", bufs=1, space="PSUM") as ps_o,
        ):
            # ---- constants ----
            ones_f1 = const.tile([1, 128], f32)
            nc.vector.memset(ones_f1[:], 1.0)
            # one-hot selector (host constant): sel[q, b*128+m] = (q == b);
            # sel_b.T @ rows[BPC, N] broadcasts rows[b] across 128 partitions
            sel_f = const.tile([BPC, BPC * 128], f32)
            nc.sync.dma_start(sel_f[:], sel[:])
            sel_sb = const.tile([BPC, BPC * 128], f32r)
            nc.vector.tensor_copy(sel_sb[:], sel_f[:])

            # ---- lens rows first: they gate the ends/starts scan ----
            lens_i = small.tile([BPC, SL], i32)
            nc.sync.dma_start(lens_i[:], lens[:])

            # ---- lens: ends/starts rows (f32r) ----
            lensf = small.tile([BPC, SL], f32)
            nc.vector.tensor_copy(lensf[:], lens_i[:])
            ends_r = small.tile([BPC, SL], f32r)
            nc.vector.tensor_tensor_scan(out=ends_r[:], data0=lensf[:], data1=lensf[:], initial=0.0, op0=Alu.add, op1=Alu.bypass)
            starts_r = small.tile([BPC, SL], f32r)
            nc.vector.tensor_sub(starts_r[:], ends_r[:], lensf[:])

            # ---- softmax(mix_weights) * gamma, broadcast to [128, NL] ----
            mw_sb = small.tile([1, NL], f32)
            nc.sync.dma_start(mw_sb[:], mw[:])
            gam_sb = small.tile([1, 1], f32)
            nc.sync.dma_start(gam_sb[:], gam[:])
            mmax = small.tile([1, 1], f32)
            nc.vector.tensor_reduce(out=mmax[:], in_=mw_sb[:], axis=Axis.X, op=Alu.max)
            nmax = small.tile([1, 1], f32)
            nc.vector.tensor_scalar(out=nmax[:], in0=mmax[:], scalar1=-1.0, scalar2=None, op0=Alu.mult)
            mexp = small.tile([1, NL], f32)
            nc.scalar.activation(out=mexp[:], in_=mw_sb[:], func=Act.Exp, bias=nmax[:], scale=1.0)
            msum = small.tile([1, 1], f32)
            nc.vector.tensor_reduce(out=msum[:], in_=mexp[:], axis=Axis.X, op=Alu.add)
            mrec = small.tile([1, 1], f32)
            nc.vector.reciprocal(out=mrec[:], in_=msum[:])
            w_row = small.tile([1, NL], f32)
            nc.vector.tensor_scalar(out=w_row[:], in0=mexp[:], scalar1=mrec[:], scalar2=gam_sb[:], op0=Alu.mult, op1=Alu.mult)
            ps_w = ps_o.tile([128, NL], f32, tag="po")
            nc.tensor.matmul(out=ps_w[:], lhsT=ones_f1[:], rhs=w_row[:], start=True, stop=True)
            w_sb = small.tile([128, NL], f32)
            nc.scalar.copy(w_sb[:], ps_w[:])

            # ---- per-position ids: cs[part, c] = 128c + part + 1 ----
            cs_i = small.tile([128, CH], i32)
            nc.gpsimd.iota(cs_i[:], pattern=[[128, CH]], base=1, channel_multiplier=1)
            cs_sb = small.tile([128, CH], f32)
            nc.vector.tensor_copy(cs_sb[:], cs_i[:])

            # ---- membership matrices for ALL examples up front ----
            # eq path: M = w0 * membership (w0 folded into the m2 build)
            Mts = []
            for b in range(BPC):
                ps_se = ps_b.tile([128, 2 * SL], f32, tag="se")
                sel_b = sel_sb[:, b * 128:(b + 1) * 128]
                nc.tensor.matmul(out=ps_se[:, 0:SL], lhsT=sel_b, rhs=starts_r[:], start=True, stop=True)
                nc.tensor.matmul(out=ps_se[:, SL:2 * SL], lhsT=sel_b, rhs=ends_r[:], start=True, stop=True)
                se_sb = sepool.tile([128, 2 * SL], f32, tag="sesb")
                nc.scalar.copy(se_sb[:], ps_se[:])

                Mt = Mpool.tile([128, CH, SL], f32r, tag=f"M{b}", name=f"M{b}")
                for c in range(CH):
                    j0 = bands[c]
                    csc = cs_sb[:, c:c + 1]
                    m2 = mpool.tile([128, SL], f32, tag="m2")
                    if eq:
                        nc.vector.tensor_scalar(
                            out=m2[:, j0:], in0=se_sb[:, SL + j0:2 * SL], scalar1=csc,
                            scalar2=w_sb[:, 0:1], op0=Alu.is_ge, op1=Alu.mult)
                    else:
                        nc.vector.tensor_scalar(
                            out=m2[:, j0:], in0=se_sb[:, SL + j0:2 * SL], scalar1=csc,
                            scalar2=None, op0=Alu.is_ge)
                    nc.vector.scalar_tensor_tensor(
                        out=Mt[:, c, j0:], in0=se_sb[:, j0:SL], scalar=csc,
                        in1=m2[:, j0:], op0=Alu.is_lt, op1=Alu.mult)
                Mts.append(Mt)

            # ---- hidden loads ----
            # eq path: two DMA-accumulate lanes per example (A: l0+l2,
            # B: l1+l3), interleaved across example pairs so the WAW
            # semaphore of the accumulating layer never stalls a queue.
            lanes = [[None, None] for _ in range(BPC)]

            def emit_hid(b, l, accum):
                ln = l % 2
                if lanes[b][ln] is None:
                    lanes[b][ln] = hpool.tile([128, CH, H], f32r, tag=f"h{ln}", name=f"h{b}_{ln}")
                ht = lanes[b][ln]
                kw = {"accum_op": Alu.add} if accum else {}
                if CHf > 0:
                    nc.gpsimd.dma_start(
                        ht[:, 0:CHf, :],
                        hid[l, b, 0:128 * CHf, :].rearrange("(c p) d -> p c d", p=128),
                        **kw)
                if CHf < CH:
                    nc.gpsimd.dma_start(
                        ht[0:P, CH - 1, :],
                        hid[l, b, 128 * (CH - 1):T, :],
                        **kw)

            def emit_tail_loads():
                # deferred low-priority loads: emitted after the leading hid
                # descgen so the big DMAs start first
                projT_f = const.tile([128, HC, NOUT], f32)
                nc.gpsimd.dma_start(projT_f[:], projT.rearrange("(i p) o -> p i o", p=128))
                projT_sb = const.tile([128, HC, NOUT], bf16)
                nc.vector.tensor_copy(projT_sb[:], projT_f[:])
                lensc_i = small.tile([128, JC, BPC], i32)
                for jh in range(JC):
                    nc.gpsimd.dma_start(lensc_i[:, jh, :], lens[:, jh * 128:(jh + 1) * 128].rearrange("b p -> p b"))
                lensc_f = small.tile([128, JC, BPC], f32)
                nc.vector.tensor_copy(lensc_f[:], lensc_i[:])
                lensc_m = small.tile([128, JC, BPC], f32)
                nc.vector.tensor_scalar_max(lensc_m[:], lensc_f[:], 1.0)
                invcnt = small.tile([128, JC, BPC], f32)
                nc.vector.reciprocal(out=invcnt[:], in_=lensc_m[:])
                return projT_sb, invcnt

            if eq:
                # pairs (0,1) then (2,3): [b]l0 [b]l1 [b+1]l0 [b+1]l1 then accums
                for b0 in range(0, BPC, 2):
                    for b in (b0, b0 + 1):
                        emit_hid(b, 0, False)
                        emit_hid(b, 1, False)
                    if b0 == 0:
                        projT_sb, invcnt = emit_tail_loads()
                    for b in (b0, b0 + 1):
                        emit_hid(b, 2, True)
                        emit_hid(b, 3, True)
            else:
                for b in range(BPC):
                    for l in range(NL):
                        ht = hpool.tile([128, CH, H], f32r, tag=f"g{l}", name=f"g{b}_{l}")
                        if CHf > 0:
                            nc.gpsimd.dma_start(
                                ht[:, 0:CHf, :],
                                hid[l, b, 0:128 * CHf, :].rearrange("(c p) d -> p c d", p=128))
                        if CHf < CH:
                            nc.gpsimd.dma_start(ht[0:P, CH - 1, :], hid[l, b, 128 * (CH - 1):T, :])
                        lanes[b].append(ht)
                    if b == 0:
                        projT_sb, invcnt = emit_tail_loads()

            # ---- per-example compute pipeline ----
            for b in range(BPC):
                Mt = Mts[b]
                if eq:
                    mms = [lanes[b][0], lanes[b][1]]
                else:
                    # premix: mixed = sum_l w[l] * hid[l] (DVE)
                    hts = lanes[b][2:]
                    mx = accpool.tile([128, CH, H], f32r, tag="mx", name="mx")
                    prev = None
                    for l in range(NL):
                        dst = mx if l == NL - 1 else accpool.tile([128, CH, H], f32, tag="acc")
                        wl = w_sb[:, l:l + 1]
                        if l == 0:
                            nc.vector.tensor_scalar(
                                out=dst[:, 0:CHf, :], in0=hts[l][:, 0:CHf, :],
                                scalar1=wl, scalar2=None, op0=Alu.mult)
                            if CHf < CH:
                                nc.vector.tensor_scalar(
                                    out=dst[0:P, CH - 1, :], in0=hts[l][0:P, CH - 1, :],
                                    scalar1=w_sb[0:P, l:l + 1], scalar2=None, op0=Alu.mult)
                        else:
                            nc.vector.scalar_tensor_tensor(
                                out=dst[:, 0:CHf, :], in0=hts[l][:, 0:CHf, :],
                                scalar=wl, in1=prev[:, 0:CHf, :], op0=Alu.mult, op1=Alu.add)
                            if CHf < CH:
                                nc.vector.scalar_tensor_tensor(
                                    out=dst[0:P, CH - 1, :], in0=hts[l][0:P, CH - 1, :],
                                    scalar=w_sb[0:P, l:l + 1], in1=prev[0:P, CH - 1, :],
                                    op0=Alu.mult, op1=Alu.add)
                        prev = dst
                    mms = [mx]

                # ---- ragged mean-pool: pooledT[h, j] += lane_c^T @ M_c ----
                # one PSUM bank per h-slice: interleaved accumulation groups
                # are only correct across different banks (HW-verified)
                pps = []
                for i in range(HC):
                    pp_i = ps_p.tile([128, SL], f32, tag=f"pp{i}", name=f"pp{i}")
                    pps.append(pp_i)
                nmm = len(mms)
                for c in range(CH):
                    pc = 128 if c < CH - 1 else P
                    j0 = bands[c]
                    for i in range(HC):
                        for k, mm in enumerate(mms):
                            nc.tensor.matmul(
                                out=pps[i][:, j0:],
                                lhsT=mm[0:pc, c, i * 128:(i + 1) * 128],
                                rhs=Mt[0:pc, c, j0:],
                                start=(c == 0 and k == 0),
                                stop=(c == CH - 1 and k == nmm - 1),
                                skip_group_check=True,
                            )
                ptsb = ptpool.tile([128, HC, SL], bf16, tag="pt")
                for i in range(HC):
                    nc.scalar.copy(ptsb[:, i, :], pps[i][:])

                # projection (bf16) + 1/cnt scale on the PSUM->SBUF copy
                for jh in range(JC):
                    po = ps_o.tile([128, NOUT], f32, tag="po")
                    for i in range(HC):
                        nc.tensor.matmul(
                            out=po[:],
                            lhsT=ptsb[:, i, jh * 128:(jh + 1) * 128],
                            rhs=projT_sb[:, i, :],
                            start=(i == 0),
                            stop=(i == HC - 1),
                        )
                    osb = opool.tile([128, NOUT], f32, tag="o")
                    nc.scalar.activation(out=osb[:], in_=po[:], func=Act.Copy, scale=invcnt[:, jh, b:b + 1])
                    nc.scalar.dma_start(out[b, jh * 128:(jh + 1) * 128, :], osb[:])

    nc.finalize()
    return nc


def _get_nc(key):
    if key not in _NC_CACHE:
        _NC_CACHE[key] = _build_nc(*key)
    return _NC_CACHE[key]


def kernel(subwords=None, bert_lens=None, bert_mask=None, hidden_states=None,
           mix_weights=None, gamma=None, proj_w=None, **_ignored):
    global LAST_RESULT
    import os
    from concourse.bass_utils import run_bass_kernel_spmd

    hs = np.asarray(hidden_states, dtype=np.float32)
    lens_np = np.asarray(bert_lens).astype(np.int32)
    mw_np = np.asarray(mix_weights, dtype=np.float32).reshape(1, NL)
    gam_np = np.asarray(gamma, dtype=np.float32).reshape(1, 1)
    projT_np = np.ascontiguousarray(np.asarray(proj_w, dtype=np.float32).T)
    sel_np = np.zeros((BPC, BPC * 128), dtype=np.float32)
    for b in range(BPC):
        sel_np[b, b * 128:(b + 1) * 128] = 1.0

    # program specialization from the runtime inputs (cached per key):
    # ragged position bound, equal-weights fast path, word-band bounds
    T = int(min(max(int(lens_np.sum(axis=1).max()), 1), SW))
    eq = bool(np.all(mw_np == mw_np.flat[0]))
    Lmax = max(int(lens_np.max()), 1)
    CH = (T + 127) // 128
    bands = tuple(max(0, min(SL - 1, -(-(128 * c + 1) // Lmax) - 1)) for c in range(CH))
    nc = _get_nc((T, eq, bands))

    in_maps = []
    for c in range(NCORES):
        sl = slice(c * BPC, (c + 1) * BPC)
        in_maps.append({
            "hid": np.ascontiguousarray(hs[:, sl, :T, :]),
            "lens": np.ascontiguousarray(lens_np[sl]),
            "mw": mw_np,
            "gam": gam_np,
            "projT": projT_np,
            "sel": sel_np,
        })

    trace = bool(int(os.environ.get("KERNEL_TRACE", "0")))
    LAST_RESULT = run_bass_kernel_spmd(nc, in_maps, list(range(NCORES)), trace=trace)
    res = LAST_RESULT.results
    return np.concatenate([r["out"] for r in res], axis=0)
